# revision 7
# baseline (speedup 1.0000x reference)
"""Int4 tensor-parallel linear for TRN2 (8 NeuronCores).

out[B,S,N] = x[B,S,K] @ dequant(weight_packed, scales).T + bias

Sharding: weight_packed/scales/bias split along N (11008 -> 8 x 1376);
x is replicated (pre-transposed on host to [K, T] so the contraction dim
lands on SBUF partitions); each core computes out[:, n_shard] and the
host concatenates.

Per-core kernel: dequantize int4 -> fp16 on DVE/ACT, transpose W via the
PE (identity matmul), then dense fp16 matmuls accumulating in PSUM.
"""

import sys

if "/opt/trn_rl_repo" not in sys.path:
    sys.path.insert(0, "/opt/trn_rl_repo")

from contextlib import ExitStack

import numpy as np

import concourse.bass as bass
import concourse.bacc as bacc
import concourse.mybir as mybir
import concourse.tile as tile
from concourse.bass_utils import run_bass_kernel_spmd
from concourse.masks import make_identity

F16 = mybir.dt.float16
F32 = mybir.dt.float32
U8 = mybir.dt.uint8

B, S, K, N = 4, 1024, 4096, 11008
T = B * S
NCORES = 8
NSH = N // NCORES


def build_kernel(
    T,
    K,
    NSH,
    TB=512,
    fuse_nibble=False,
    bias_on_dve=True,
    use_dma_transpose=False,
    xt_bufs=48,
):
    """Single-core Bass program: out[T,NSH] = xT.T @ deq(wp,sc).T + bias."""
    assert K % 256 == 0 and T % TB == 0 and TB % 128 == 0
    G = K // 128  # scale groups (group_size 128)
    KH = K // 2

    ntiles = []
    n0 = 0
    while n0 < NSH:
        ntiles.append((n0, min(128, NSH - n0)))
        n0 += 128
    chunks = []
    c0 = 0
    while c0 < NSH:
        chunks.append((c0, min(512, NSH - c0)))
        c0 += 512

    nc = bacc.Bacc("TRN2", target_bir_lowering=False, debug=False)
    xT_d = nc.dram_tensor("xT", (K, T), F16, kind="ExternalInput")
    wp_d = nc.dram_tensor("wp", (NSH, KH), U8, kind="ExternalInput")
    sc_d = nc.dram_tensor("sc", (NSH, G), F16, kind="ExternalInput")
    bias_d = nc.dram_tensor("bias", (1, NSH), F16, kind="ExternalInput")
    out_d = nc.dram_tensor("out", (T, NSH), F16, kind="ExternalOutput")

    with tile.TileContext(nc) as tc, ExitStack() as ctx:
        const_p = ctx.enter_context(tc.tile_pool(name="const", bufs=1))
        wt_p = ctx.enter_context(tc.tile_pool(name="wt", bufs=1))
        wp_p = ctx.enter_context(tc.tile_pool(name="wpk", bufs=2))
        q_p = ctx.enter_context(tc.tile_pool(name="q", bufs=2))
        wd_p = ctx.enter_context(tc.tile_pool(name="wd", bufs=2))
        xt_p = ctx.enter_context(tc.tile_pool(name="xt", bufs=xt_bufs))
        ob_p = ctx.enter_context(tc.tile_pool(name="ob", bufs=3))
        mpsum = ctx.enter_context(tc.tile_pool(name="mpsum", bufs=6, space="PSUM"))
        if not use_dma_transpose:
            tpsum = ctx.enter_context(tc.tile_pool(name="tpsum", bufs=2, space="PSUM"))

        ident = const_p.tile([128, 128], F16)
        make_identity(nc, ident[:])

        # bias broadcast tile [128, NSH] via K=1 matmul with a ones row
        bias_row = const_p.tile([1, NSH], F16)
        nc.sync.dma_start(bias_row[:], bias_d[:, :])
        ones_row = const_p.tile([1, 128], F16)
        nc.vector.memset(ones_row[:], 1.0)
        bias_b = const_p.tile([128, NSH], F16)
        for c0, csz in chunks:
            bp = mpsum.tile([128, 512], F32, tag="mp", name=f"biasb{c0}")
            nc.tensor.matmul(
                bp[:, :csz], ones_row[:], bias_row[:, c0 : c0 + csz],
                start=True, stop=True,
            )
            nc.vector.tensor_copy(bias_b[:, c0 : c0 + csz], bp[:, :csz])

        # per-group transposed weights wT[g]: [128 (k within group), NSH]
        wT = [wt_p.tile([128, NSH], F16, tag=f"wT{g}", name=f"wT{g}") for g in range(G)]

        # all scales up front into one tile (no slot reuse -> no WAR waits
        # piling onto the tiny per-n-tile DMAs)
        NT = len(ntiles)
        s_all16 = const_p.tile([128, G * NT], F16)
        nc.vector.memset(s_all16[:], 0.0)
        nfull = NSH // 128  # full 128-row n-tiles
        if nfull:
            src = sc_d[0 : nfull * 128, :].rearrange("(nt p) g -> p nt g", p=128)
            dst = s_all16[:, 0 : nfull * G].rearrange("p (nt g) -> p nt g", g=G)
            nc.sync.dma_start(dst, src)
        if NT > nfull:  # ragged tail tile
            r0, rsz = ntiles[-1]
            nc.sync.dma_start(s_all16[:rsz, nfull * G :], sc_d[r0 : r0 + rsz, :])
        s_all = const_p.tile([128, G * NT], F32)
        nc.vector.tensor_copy(s_all[:], s_all16[:])
        m8s_all = const_p.tile([128, G * NT], F32)
        nc.vector.tensor_scalar_mul(m8s_all[:], s_all[:], -8.0)

        # ---- Phase 1: dequantize + transpose W ----
        for nt, (r0, rsz) in enumerate(ntiles):
            wp_t = wp_p.tile([128, KH], U8, tag="wp")
            nc.sync.dma_start(wp_t[:rsz], wp_d[r0 : r0 + rsz, :])
            s_t = s_all[:, nt * G : (nt + 1) * G]
            m8s_t = m8s_all[:, nt * G : (nt + 1) * G]

            q_t = q_p.tile([128, K], F16, tag="q")
            if fuse_nibble:
                nc.vector.tensor_scalar(
                    q_t[:rsz, 0:K:2], wp_t[:rsz], 15, None,
                    op0=mybir.AluOpType.bitwise_and,
                )
                nc.vector.tensor_scalar(
                    q_t[:rsz, 1:K:2], wp_t[:rsz], 4, None,
                    op0=mybir.AluOpType.logical_shift_right,
                )
            else:
                lo_t = q_p.tile([128, KH], U8, tag="lo")
                hi_t = q_p.tile([128, KH], U8, tag="hi")
                nc.vector.tensor_scalar(
                    lo_t[:rsz], wp_t[:rsz], 15, None,
                    op0=mybir.AluOpType.bitwise_and,
                )
                nc.vector.tensor_scalar(
                    hi_t[:rsz], wp_t[:rsz], 4, None,
                    op0=mybir.AluOpType.logical_shift_right,
                )
                nc.vector.tensor_copy(q_t[:rsz, 0:K:2], lo_t[:rsz])
                nc.vector.tensor_copy(q_t[:rsz, 1:K:2], hi_t[:rsz])

            wd_t = wd_p.tile([128, K], F16, tag="wd")
            for g in range(G):
                # Identity(q * s + (-8 s)) == (q - 8) * s, per-partition scalars
                nc.scalar.activation(
                    wd_t[:rsz, g * 128 : (g + 1) * 128],
                    q_t[:rsz, g * 128 : (g + 1) * 128],
                    mybir.ActivationFunctionType.Identity,
                    bias=m8s_t[:rsz, g : g + 1],
                    scale=s_t[:rsz, g : g + 1],
                )
            for g in range(G):
                src = wd_t[:rsz, g * 128 : (g + 1) * 128]
                if use_dma_transpose:
                    nc.sync.dma_start_transpose(wT[g][:, r0 : r0 + rsz], src)
                else:
                    pt = tpsum.tile([128, 128], F16, tag="pt")
                    nc.tensor.transpose(pt[:, :rsz], src, ident[:rsz, :rsz])
                    nc.vector.tensor_copy(wT[g][:, r0 : r0 + rsz], pt[:, :rsz])

        # ---- Phase 2: matmul ----
        KT = K // 128
        for tb in range(T // TB):
            t0 = tb * TB
            xts = []
            for k in range(KT):
                xt_t = xt_p.tile([128, TB], F16, tag="xt")
                nc.sync.dma_start(xt_t[:], xT_d[k * 128 : (k + 1) * 128, t0 : t0 + TB])
                xts.append(xt_t)
            for ts_ in range(TB // 128):
                psums = [
                    mpsum.tile([128, 512], F32, tag="mp", name=f"mp{tb}_{ts_}_{ci}")
                    for ci in range(len(chunks))
                ]
                for k in range(KT):
                    lhsT = xts[k][:, ts_ * 128 : (ts_ + 1) * 128]
                    for ci, (c0, csz) in enumerate(chunks):
                        nc.tensor.matmul(
                            psums[ci][:, :csz], lhsT, wT[k][:, c0 : c0 + csz],
                            start=(k == 0), stop=(k == KT - 1),
                        )
                ob = ob_p.tile([128, NSH], F16, tag="ob")
                for ci, (c0, csz) in enumerate(chunks):
                    if bias_on_dve:
                        nc.vector.tensor_add(
                            ob[:, c0 : c0 + csz], psums[ci][:, :csz],
                            bias_b[:, c0 : c0 + csz],
                        )
                    else:
                        nc.vector.tensor_copy(ob[:, c0 : c0 + csz], psums[ci][:, :csz])
                row0 = t0 + ts_ * 128
                nc.sync.dma_start(out_d[row0 : row0 + 128, :], ob[:])

    nc.compile()
    return nc


_NC_CACHE = {}


def _get_nc(**kw):
    key = tuple(sorted(kw.items()))
    if key not in _NC_CACHE:
        _NC_CACHE[key] = build_kernel(T, K, NSH, **kw)
    return _NC_CACHE[key]


def _prep_in_maps(x, weight_packed, scales, bias):
    x = np.asarray(x, dtype=np.float16)
    wp = np.asarray(weight_packed)
    if wp.dtype != np.uint8:
        wp = wp.astype(np.uint8)
    sc = np.asarray(scales, dtype=np.float16)
    b = np.asarray(bias, dtype=np.float16).reshape(1, N)
    xT = np.ascontiguousarray(x.reshape(T, K).T)
    in_maps = []
    for c in range(NCORES):
        sl = slice(c * NSH, (c + 1) * NSH)
        in_maps.append(
            {
                "xT": xT,
                "wp": np.ascontiguousarray(wp[sl]),
                "sc": np.ascontiguousarray(sc[sl]),
                "bias": np.ascontiguousarray(b[:, sl]),
            }
        )
    return in_maps


def run(x, weight_packed, scales, bias, trace=False, **build_kw):
    nc = _get_nc(**build_kw)
    in_maps = _prep_in_maps(x, weight_packed, scales, bias)
    res = run_bass_kernel_spmd(
        nc, in_maps, core_ids=list(range(NCORES)), trace=trace
    )
    out = np.concatenate([r["out"] for r in res.results], axis=1)
    return out.reshape(B, S, N), res


def kernel(x, weight_packed, scales, bias, group_size=128, **_ignored):
    assert int(np.asarray(group_size)) == 128
    out, _ = run(x, weight_packed, scales, bias)
    return out
